# revision 22
# baseline (speedup 1.0000x reference)
"""BirthDeathAttention kernel for 8 Trainium2 NeuronCores.

Math note: in the reference, both `persistence_bias` ([1,H,1,1]) and
`importance_weights[:, None, :, None] * 0.1` ([B,1,N,1]) are constant along
the softmax (key) axis, so they cancel exactly inside the softmax.  The
module is therefore plain multi-head attention + output projection.

Sharding (per the tensor-parallel hint): core = (batch b, head-group g),
b in {0,1}, g in {0..3}, each core handling 4 of the 16 heads for one batch
element.  Each core computes a partial output projection (its heads' slice
of W_proj rows); the host sums the 4 partials per batch and adds b_proj.

Per-core schedule: the kernel is jointly limited by ScalarE (128 exp
activations over [128,1024] score tiles ~ 137us) and the PE (~137us of
matmuls), so the whole design aims at keeping the exp stream back-to-back
while PE work (qkv projection chains A/B, scores S, attention@v U, output
projection E) drains in the gaps:

  - 8 "waves", one per (query-block, head-pair), ordered
    (0,0),(1,0),(0,1),(1,1),(2,0),(3,0),(2,1),(3,1) so the A-chain and
    E-block work spreads across the exp-bound steady state.
  - psS pool (2x2 banks) score tiles; exp paces S via slot reuse.
  - psU pool (2x1 banks) U accumulators, eagerly evicted to SBUF so the
    banks free fast; softmax denominator rides as v's ones column (row 64).
  - psAB pool (2x1 banks) for A/B/E chains so they interleave anywhere.
  - v stationary padded to a 128-column window so FWL keeps LDWEIGHTS
    off the critical path in the U matmuls.
  - normalization: reciprocal on [1,512] rows, DRAM-roundtrip broadcast
    (off critical path), DVE multiplies.
  - input DMAs batched (few big transfers, sync+gpsimd queues); ACT exp
    table prewarmed by a dummy activation at t=0.
"""

import sys

if "/opt/trn_rl_repo" not in sys.path:
    sys.path.insert(0, "/opt/trn_rl_repo")

import numpy as np
import ml_dtypes

import concourse.bass as bass
import concourse.mybir as mybir
import concourse.tile as tile
from concourse.bass_utils import run_bass_kernel_spmd

DIM = 1024
N = 2048
B = 2
HEADS = 16
HEAD_DIM = 64
SCALE = HEAD_DIM ** -0.5
HPG = 4          # heads per group (per core)
GC = HPG * HEAD_DIM  # channels per core = 256
BF16 = mybir.dt.bfloat16
F32 = mybir.dt.float32

KT = DIM // 128      # 8 contraction tiles over model dim
NB = N // 512        # 4 query blocks
NKT = N // 128       # 16 key tiles
VPAD = 3 * 65 + 128  # v free size: 4 heads @ stride 65 + M=128 window pad


def _split_multi_waits(nc, max_waits=1):
    """The walrus build in this container accepts at most one sync-wait per
    instruction.  Hoist extra waits onto single-wait NOPs inserted just
    before the instruction in its engine's program order."""
    uid = [0]
    for f in nc.m.functions:
        for bb in f.blocks:
            insts = bb.instructions
            new = []
            changed = False
            for inst in insts:
                si = inst.sync_info
                if si is not None and len(si.on_wait) > max_waits:
                    waits = list(si.on_wait)
                    for w in waits[:-max_waits]:
                        nop = mybir.InstNoOp(
                            name=f"I-splitw-{uid[0]}", ins=[], outs=[])
                        uid[0] += 1
                        nop.engine = inst.engine
                        nop.sync_info = mybir.SyncInfo(
                            on_wait=[w], on_update=[])
                        new.append(nop)
                    si.on_wait = waits[-max_waits:]
                    inst.sync_info = si
                    changed = True
                new.append(inst)
            if changed:
                bb.instructions = new


def build_core_kernel() -> bass.Bass:
    nc = bass.Bass()
    xT = nc.declare_dram_parameter("xT", [DIM, N], BF16, isOutput=False)
    wqk = nc.declare_dram_parameter("wqk", [DIM, 2 * GC], BF16, isOutput=False)
    wv = nc.declare_dram_parameter("wv", [DIM, GC], BF16, isOutput=False)
    wp = nc.declare_dram_parameter("wp", [GC, DIM], BF16, isOutput=False)
    out = nc.declare_dram_parameter("out", [N, DIM], BF16, isOutput=True)

    xT_r = xT.rearrange("(kt p) n -> p kt n", p=128)
    wqk_r = wqk.rearrange("(kt p) c -> p kt c", p=128)
    wv_r = wv.rearrange("(kt p) c -> p kt c", p=128)
    wp_r = wp.rearrange("(pair p) c -> p pair c", p=128)

    with tile.TileContext(nc) as tc:
        from contextlib import ExitStack

        with ExitStack() as ctx:
            consts = ctx.enter_context(tc.tile_pool(name="consts", bufs=1))
            sbuf = ctx.enter_context(tc.tile_pool(name="sbuf", bufs=1))

            # --- resident SBUF tensors -------------------------------------
            xT_sb = sbuf.tile([128, KT, N], BF16, tag="xT")
            wqk_sb = consts.tile([128, KT, 2 * GC], BF16, tag="wqk")
            wv_sb = consts.tile([128, KT, GC], BF16, tag="wv")
            wp_sb = consts.tile([128, 2, DIM], BF16, tag="wp")
            warm_sb = consts.tile([128, 16], BF16, tag="warm")
            qk_sb = sbuf.tile([128, 4, N], BF16, tag="qk")
            # v with a ones column appended per head ([v_h | 1], stride 65):
            # the ones column turns attention@v into a matmul that also
            # emits the softmax denominator as output row 64.  Free size
            # padded so every head has a 128-col stationary window (keeps
            # FWL active -> LDWEIGHTS backgrounded).
            v_sb = sbuf.tile([128, NKT, VPAD], BF16, tag="v")
            o_sb = sbuf.tile([128, 2, N], BF16, tag="o")

            # ACT exp-table prewarm: runs at t=0, hides the ~2.7us
            # PSEUDO_LOAD_ACT_FUNC_SET under the input DMAs.
            nc.vector.memset(warm_sb[:], 1.0)
            nc.scalar.activation(
                warm_sb[:], warm_sb[:],
                mybir.ActivationFunctionType.Exp, scale=1.0)
            nc.vector.memset(v_sb[:], 1.0)

            # select matrix for the denominator broadcast matmul:
            # out[m, q] = r[0, q] for m < 64 else r[32, q]
            sel_sb = consts.tile([33, 128], BF16, tag="sel")
            nc.vector.memset(sel_sb[:], 0.0)
            nc.vector.memset(sel_sb[0:1, 0:64], 1.0)
            nc.vector.memset(sel_sb[32:33, 64:128], 1.0)
            # persistent double-buffered denominator/reciprocal tiles
            # (init to 1.0 so the unused rows never hold NaN/Inf)
            d_rt = sbuf.tile([33, 2, 512], F32, tag="d")
            r_rt = sbuf.tile([33, 2, 512], F32, tag="r")
            rb_rt = sbuf.tile([33, 2, 512], BF16, tag="rb")
            nc.vector.memset(d_rt[:], 1.0)
            nc.vector.memset(r_rt[:], 1.0)
            nc.vector.memset(rb_rt[:], 1.0)

            # --- batched input DMAs ----------------------------------------
            # issue in need-order: wv first (v chains are the early PE
            # filler), then wave 0's wqk slices (k-pair0 ct=2, q-pair0
            # ct=0) + x block 0.  Transfers parallelize across the 16 DMA
            # engines regardless of queue.
            nc.sync.dma_start(out=wv_sb[:], in_=wv_r[:])
            for ct in (2, 0, 3, 1):
                nc.sync.dma_start(
                    out=wqk_sb[:, :, ct * 128:(ct + 1) * 128],
                    in_=wqk_r[:, :, ct * 128:(ct + 1) * 128],
                )
                if ct == 2:
                    nc.sync.dma_start(
                        out=xT_sb[:, :, 0:512], in_=xT_r[:, :, 0:512])
            for nb in range(1, NB):
                nc.sync.dma_start(
                    out=xT_sb[:, :, nb * 512:(nb + 1) * 512],
                    in_=xT_r[:, :, nb * 512:(nb + 1) * 512],
                )
            nc.sync.dma_start(out=wp_sb[:], in_=wp_r[:])

            # --- helper emitters -------------------------------------------
            def a_chain(pool, ct, nb):
                acc = pool.tile([128, 512], F32, tag="ab")
                for kt in range(KT):
                    nc.tensor.matmul(
                        acc[:],
                        lhsT=wqk_sb[:, kt, ct * 128:(ct + 1) * 128],
                        rhs=xT_sb[:, kt, nb * 512:(nb + 1) * 512],
                        start=(kt == 0),
                        stop=(kt == KT - 1),
                    )
                nc.vector.tensor_copy(
                    qk_sb[:, ct, nb * 512:(nb + 1) * 512], acc[:]
                )

            def b_chain(pool, nt):
                acc = pool.tile([128, 512], F32, tag="ab")
                for kt in range(KT):
                    nc.tensor.matmul(
                        acc[:, 0:GC],
                        lhsT=xT_sb[:, kt, nt * 128:(nt + 1) * 128],
                        rhs=wv_sb[:, kt, :],
                        start=(kt == 0),
                        stop=(kt == KT - 1),
                    )
                for h in range(HPG):
                    nc.vector.tensor_copy(
                        v_sb[:, nt, h * 65:h * 65 + 64],
                        acc[:, h * 64:(h + 1) * 64],
                    )

            def s_exp_block(psS, nqb, pair, e_t, nkts=None):
                qt = qk_sb[:, pair, :]
                kt_sb = qk_sb[:, 2 + pair, :]
                for nkt in (range(NKT) if nkts is None else nkts):
                    st = psS.tile([128, 1024], F32, tag="st")
                    for hh in range(2):
                        nc.tensor.matmul(
                            st[:, hh * 512:(hh + 1) * 512],
                            lhsT=kt_sb[
                                hh * 64:(hh + 1) * 64,
                                nkt * 128:(nkt + 1) * 128,
                            ],
                            rhs=qt[
                                hh * 64:(hh + 1) * 64,
                                nqb * 512:(nqb + 1) * 512,
                            ],
                            start=True,
                            stop=True,
                        )
                    nc.scalar.activation(
                        e_t[:, nkt, :],
                        st[:],
                        mybir.ActivationFunctionType.Exp,
                        scale=SCALE,
                    )

            def u_quarter(u_a, u_b, pair, e_t, nkts):
                # U_aug = [v|1]^T E^T per head with a padded 128-col
                # stationary window: rows 0-63 the head's v, row 64 the
                # softmax denominator, rows 65+ garbage (ignored).
                for nkt in nkts:
                    for hh, u_t in ((0, u_a), (1, u_b)):
                        h = pair * 2 + hh
                        nc.tensor.matmul(
                            u_t[:],
                            lhsT=v_sb[:, nkt, h * 65:h * 65 + 128],
                            rhs=e_t[:, nkt, hh * 512:(hh + 1) * 512],
                            start=(nkt == 0),
                            stop=(nkt == NKT - 1),
                        )

            def norm_part1(u_a, u_b, upool, k):
                # eager eviction: move the useful rows to SBUF so the PSUM
                # banks free immediately.  SBUF operands of a DVE op must
                # share the start partition and be 32-aligned, so head b
                # lands at partitions 64-127 and the denominators at rows
                # 0/32 of the double-buffered [33,2,512] tiles.
                u_sb = upool.tile([128, 512], F32, tag="usb")
                nc.vector.tensor_copy(u_sb[0:64, :], u_a[0:64, :])
                nc.vector.tensor_copy(u_sb[64:128, :], u_b[0:64, :])
                s = k % 2
                nc.vector.tensor_copy(d_rt[0:1, s, :], u_a[64:65, :])
                nc.vector.tensor_copy(d_rt[32:33, s, :], u_b[64:65, :])
                nc.vector.reciprocal(r_rt[:, s, :], d_rt[:, s, :])
                nc.vector.tensor_copy(rb_rt[:, s, :], r_rt[:, s, :])
                return u_sb

            def norm_part2(u_sb, pool, k, nqb, pair):
                # broadcast 1 -> 64 partitions with a tiny select matmul
                # (K=33: rows 1-31 of sel are zero, r rows 1-31 are 1.0).
                # Emitted one iteration after part1 so the PE FIFO never
                # waits on the 3.3us reciprocal.
                s = k % 2
                rr_ps = pool.tile([128, 512], F32, tag="ab")
                nc.tensor.matmul(
                    rr_ps[:], lhsT=sel_sb[:], rhs=rb_rt[:, s, :],
                    start=True, stop=True,
                )
                nc.vector.tensor_mul(
                    o_sb[0:64, pair, nqb * 512:(nqb + 1) * 512],
                    u_sb[0:64, :],
                    rr_ps[0:64, :],
                )
                nc.vector.tensor_mul(
                    o_sb[64:128, pair, nqb * 512:(nqb + 1) * 512],
                    u_sb[64:128, :],
                    rr_ps[64:128, :],
                )

            def e_block(pool, opool, nqb, mts=None):
                # partial output projection for query block nqb
                for mt in (range(nqb * 4, nqb * 4 + 4) if mts is None
                           else mts):
                    ot = opool.tile([128, DIM], BF16, tag="ot")
                    for nh in range(2):
                        acc = pool.tile([128, 512], F32, tag="ab")
                        for pair in range(2):
                            nc.tensor.matmul(
                                acc[:],
                                lhsT=o_sb[:, pair, mt * 128:(mt + 1) * 128],
                                rhs=wp_sb[:, pair, nh * 512:(nh + 1) * 512],
                                start=(pair == 0),
                                stop=(pair == 1),
                            )
                        nc.vector.tensor_copy(
                            ot[:, nh * 512:(nh + 1) * 512], acc[:]
                        )
                    nc.sync.dma_start(
                        out=out[mt * 128:(mt + 1) * 128, :], in_=ot[:]
                    )

            # --- wave schedule ---------------------------------------------
            WAVES = [(0, 0), (1, 0), (0, 1), (1, 1),
                     (2, 0), (3, 0), (2, 1), (3, 1)]

            with (
                tc.tile_pool(name="psS", bufs=2, space="PSUM") as psS,
                tc.tile_pool(name="psU", bufs=2, space="PSUM") as psU,
                tc.tile_pool(name="psAB", bufs=2, space="PSUM") as psAB,
                tc.tile_pool(name="epool", bufs=3) as epool,
                tc.tile_pool(name="upool", bufs=2) as upool,
                tc.tile_pool(name="opool", bufs=3) as opool,
            ):
                e_q = []

                def new_e():
                    e_t = epool.tile([128, NKT, 1024], BF16, tag="e")
                    e_q.append(e_t)
                    return e_t

                # prologue: k(pair0) + q(block0) chains slide under wave 0;
                # v chains threaded through as the early PE filler.
                a_chain(psAB, 2, 0)
                a_chain(psAB, 0, 0)
                e0 = new_e()
                s_exp_block(psS, 0, 0, e0, nkts=range(0, 4))
                for nb in range(1, NB):
                    a_chain(psAB, 2, nb)
                    s_exp_block(psS, 0, 0, e0, nkts=range(nb * 4, nb * 4 + 4))
                a_chain(psAB, 0, 1)
                b_chain(psAB, 0)
                # wave 1 quarters with k-pair1 + v fillers
                e1_ = new_e()
                w1f = [[lambda: b_chain(psAB, 1)],
                       [lambda: a_chain(psAB, 3, 0),
                        lambda: a_chain(psAB, 3, 1)],
                       [lambda: b_chain(psAB, 2)],
                       [lambda: b_chain(psAB, 3),
                        lambda: a_chain(psAB, 3, 2)]]
                for q in range(4):
                    s_exp_block(psS, 1, 0, e1_, nkts=range(q * 4, q * 4 + 4))
                    for f in w1f[q]:
                        f()
                a_chain(psAB, 3, 3)
                a_chain(psAB, 1, 0)
                # wave 2 quarters with the bulk of the v chains
                e2_ = new_e()
                w2f = [[4, 5], [6, 7], [8], [9]]
                for q in range(4):
                    s_exp_block(psS, 0, 1, e2_, nkts=range(q * 4, q * 4 + 4))
                    for nt in w2f[q]:
                        b_chain(psAB, nt)
                    if q == 2:
                        a_chain(psAB, 1, 1)

                # per-iteration fillers, spread per quarter q (emitted
                # after that quarter's U matmuls, before its S wave).
                # b(10..15) land before the u quarters that read them;
                # E(nqb) goes after both its pairs' norms.
                fillers = {
                    (0, 0): [lambda: b_chain(psAB, 10),
                             lambda: b_chain(psAB, 11)],
                    (0, 1): [lambda: b_chain(psAB, 12),
                             lambda: b_chain(psAB, 13)],
                    (0, 2): [lambda: b_chain(psAB, 14),
                             lambda: b_chain(psAB, 15)],
                    (1, 0): [lambda: a_chain(psAB, 0, 2)],
                    (2, 0): [lambda: a_chain(psAB, 0, 3)],
                    (2, 2): [lambda: a_chain(psAB, 1, 2)],
                    (3, 0): [lambda: a_chain(psAB, 1, 3)],
                    (3, 1): [lambda: e_block(psAB, opool, 0,
                                             mts=range(0, 2))],
                    (3, 3): [lambda: e_block(psAB, opool, 0,
                                             mts=range(2, 4))],
                    (4, 0): [lambda: e_block(psAB, opool, 1,
                                             mts=range(4, 5))],
                    (4, 1): [lambda: e_block(psAB, opool, 1,
                                             mts=range(5, 6))],
                    (4, 2): [lambda: e_block(psAB, opool, 1,
                                             mts=range(6, 7))],
                    (4, 3): [lambda: e_block(psAB, opool, 1,
                                             mts=range(7, 8))],
                    (7, 0): [lambda: e_block(psAB, opool, 2,
                                             mts=range(8, 10))],
                    (7, 1): [lambda: e_block(psAB, opool, 2,
                                             mts=range(10, 11))],
                    (7, 2): [lambda: e_block(psAB, opool, 2,
                                             mts=range(11, 12))],
                }
                u_sb_prev = None
                for k in range(8):
                    nqb, pair = WAVES[k]
                    if u_sb_prev is not None:
                        pq, pp = WAVES[k - 1]
                        norm_part2(u_sb_prev, psAB, k - 1, pq, pp)
                    u_a = psU.tile([128, 512], F32, tag="u")
                    u_b = psU.tile([128, 512], F32, tag="u")
                    if k + 3 < 8:
                        e_next = new_e()
                        nq2, p2 = WAVES[k + 3]
                    for q in range(4):
                        # b(10..15) must precede the u quarters that read
                        # them: q's U reads v tiles 4q..4q+3, so emit the
                        # fillers of quarter q before u of quarter q+1.
                        u_quarter(u_a, u_b, pair, e_q[k],
                                  range(q * 4, q * 4 + 4))
                        for f in fillers.get((k, q), ()):
                            f()
                        if k + 3 < 8:
                            s_exp_block(psS, nq2, p2, e_next,
                                        nkts=range(q * 4, q * 4 + 4))
                    u_sb_prev = norm_part1(u_a, u_b, upool, k)
                norm_part2(u_sb_prev, psAB, 7, *WAVES[7])
                e_block(psAB, opool, 3)

    _split_multi_waits(nc)
    return nc


_NC_CACHE = None


def _get_nc():
    global _NC_CACHE
    if _NC_CACHE is None:
        _NC_CACHE = build_core_kernel()
    return _NC_CACHE


def kernel(x, importance_weights, W_qkv, W_proj, b_proj, persistence_bias,
           _results_hook=None):
    x = np.asarray(x)
    W_qkv = np.asarray(W_qkv, dtype=np.float32)
    W_proj = np.asarray(W_proj, dtype=np.float32)
    b_proj = np.asarray(b_proj, dtype=np.float32)

    bf = ml_dtypes.bfloat16
    Q = W_qkv[:, 0:DIM]
    K = W_qkv[:, DIM:2 * DIM]
    V = W_qkv[:, 2 * DIM:3 * DIM]

    in_maps = []
    for core in range(8):
        b, g = divmod(core, 4)
        sl = slice(g * GC, (g + 1) * GC)
        in_maps.append({
            "xT": np.ascontiguousarray(x[b].T).astype(bf),
            "wqk": np.ascontiguousarray(
                np.concatenate([Q[:, sl], K[:, sl]], axis=1)).astype(bf),
            "wv": np.ascontiguousarray(V[:, sl]).astype(bf),
            "wp": np.ascontiguousarray(W_proj[sl, :]).astype(bf),
        })

    nc = _get_nc()
    res = run_bass_kernel_spmd(nc, in_maps, list(range(8)))
    if _results_hook is not None:
        _results_hook(res)

    out = np.zeros((B, N, DIM), dtype=np.float32)
    for core in range(8):
        b = core // 4
        out[b] += res.results[core]["out"].astype(np.float32)
    out += b_proj[None, None, :]
    return out


# revision 31
# speedup vs baseline: 1.1253x; 1.1253x over previous
"""BirthDeathAttention kernel for 8 Trainium2 NeuronCores.

Math note: in the reference, both `persistence_bias` ([1,H,1,1]) and
`importance_weights[:, None, :, None] * 0.1` ([B,1,N,1]) are constant along
the softmax (key) axis, so they cancel exactly inside the softmax.  The
module is therefore plain multi-head attention + output projection.

Sharding (per the tensor-parallel hint): core = (batch b, head-group g),
b in {0,1}, g in {0..3}, each core handling 4 of the 16 heads for one batch
element.  Each core computes a partial output projection (its heads' slice
of W_proj rows); the host sums the 4 partials per batch and adds b_proj.

Per-core schedule: the kernel is jointly limited by ScalarE (128 exp
activations over [128,1024] score tiles ~ 137us) and the PE (~137us of
matmuls), so the whole design aims at keeping the exp stream back-to-back
while PE work (qkv projection chains A/B, scores S, attention@v U, output
projection E) drains in the gaps:

  - 8 "waves", one per (query-block, head-pair), ordered
    (0,0),(1,0),(0,1),(1,1),(2,0),(3,0),(2,1),(3,1) so the A-chain and
    E-block work spreads across the exp-bound steady state.
  - psS pool (2x2 banks) score tiles; exp paces S via slot reuse.
  - psU pool (2x1 banks) U accumulators, eagerly evicted to SBUF so the
    banks free fast; softmax denominator rides as v's ones column (row 64).
  - psAB pool (2x1 banks) for A/B/E chains so they interleave anywhere.
  - v stationary padded to a 128-column window so FWL keeps LDWEIGHTS
    off the critical path in the U matmuls.
  - normalization: reciprocal on [1,512] rows, DRAM-roundtrip broadcast
    (off critical path), DVE multiplies.
  - input DMAs batched (few big transfers, sync+gpsimd queues); ACT exp
    table prewarmed by a dummy activation at t=0.
"""

import sys

if "/opt/trn_rl_repo" not in sys.path:
    sys.path.insert(0, "/opt/trn_rl_repo")

import numpy as np
import ml_dtypes

import concourse.bass as bass
import concourse.mybir as mybir
import concourse.tile as tile
from concourse.bass_utils import run_bass_kernel_spmd

DIM = 1024
N = 2048
B = 2
HEADS = 16
HEAD_DIM = 64
SCALE = HEAD_DIM ** -0.5
HPG = 4          # heads per group (per core)
GC = HPG * HEAD_DIM  # channels per core = 256
BF16 = mybir.dt.bfloat16
F32 = mybir.dt.float32

KT = DIM // 128      # 8 contraction tiles over model dim
NB = N // 512        # 4 query blocks
NKT = N // 128       # 16 key tiles
VPAD = 3 * 65 + 128  # v free size: 4 heads @ stride 65 + M=128 window pad


def _split_multi_waits(nc, max_waits=1):
    """The walrus build in this container accepts at most one sync-wait per
    instruction.  Hoist extra waits onto single-wait NOPs inserted just
    before the instruction in its engine's program order."""
    uid = [0]
    for f in nc.m.functions:
        for bb in f.blocks:
            insts = bb.instructions
            new = []
            changed = False
            for inst in insts:
                si = inst.sync_info
                if si is not None and len(si.on_wait) > max_waits:
                    waits = list(si.on_wait)
                    for w in waits[:-max_waits]:
                        nop = mybir.InstNoOp(
                            name=f"I-splitw-{uid[0]}", ins=[], outs=[])
                        uid[0] += 1
                        nop.engine = inst.engine
                        nop.sync_info = mybir.SyncInfo(
                            on_wait=[w], on_update=[])
                        new.append(nop)
                    si.on_wait = waits[-max_waits:]
                    inst.sync_info = si
                    changed = True
                new.append(inst)
            if changed:
                bb.instructions = new


def build_core_kernel() -> bass.Bass:
    nc = bass.Bass()
    xT = nc.declare_dram_parameter("xT", [DIM, N], BF16, isOutput=False)
    wqk = nc.declare_dram_parameter("wqk", [DIM, 2 * GC], BF16, isOutput=False)
    wv = nc.declare_dram_parameter("wv", [DIM, GC], BF16, isOutput=False)
    wp = nc.declare_dram_parameter("wp", [GC, DIM], BF16, isOutput=False)
    out = nc.declare_dram_parameter("out", [N, DIM], BF16, isOutput=True)

    xT_r = xT.rearrange("(kt p) n -> p kt n", p=128)
    wqk_r = wqk.rearrange("(kt p) c -> p kt c", p=128)
    wv_r = wv.rearrange("(kt p) c -> p kt c", p=128)
    wp_r = wp.rearrange("(pair p) c -> p pair c", p=128)

    with tile.TileContext(nc) as tc:
        from contextlib import ExitStack

        with ExitStack() as ctx:
            consts = ctx.enter_context(tc.tile_pool(name="consts", bufs=1))
            sbuf = ctx.enter_context(tc.tile_pool(name="sbuf", bufs=1))

            # --- resident SBUF tensors -------------------------------------
            xT_sb = sbuf.tile([128, KT, N], BF16, tag="xT")
            wqk_sb = consts.tile([128, KT, 2 * GC], BF16, tag="wqk")
            wv_sb = consts.tile([128, KT, GC], BF16, tag="wv")
            wp_sb = consts.tile([128, 2, DIM], BF16, tag="wp")
            warm_sb = consts.tile([128, 16], BF16, tag="warm")
            qk_sb = sbuf.tile([128, 4, N], BF16, tag="qk")
            # v with a ones column appended per head ([v_h | 1], stride 65):
            # the ones column turns attention@v into a matmul that also
            # emits the softmax denominator as output row 64.  Free size
            # padded so every head has a 128-col stationary window (keeps
            # FWL active -> LDWEIGHTS backgrounded).
            v_sb = sbuf.tile([128, NKT, VPAD], BF16, tag="v")
            o_sb = sbuf.tile([128, 2, N], BF16, tag="o")

            # ACT exp-table prewarm: runs at t=0, hides the ~2.7us
            # PSEUDO_LOAD_ACT_FUNC_SET under the input DMAs.  (warm_sb is
            # read uninitialized — exp of garbage is fine, output unused.)
            nc.scalar.activation(
                warm_sb[:], warm_sb[:],
                mybir.ActivationFunctionType.Exp, scale=0.0)

            # --- batched input DMAs ----------------------------------------
            # wave 0 needs wqk's k-pair0 slice (ct=2) + q-pair0 (ct=0) + x
            # block 0 first; kt-chunked so the first chains slide with the
            # arriving data.  Issues spread across sync/vector/scalar
            # queues (transfers parallelize across the 16 DMA engines).
            def dma_wqk(eng, ct, k0, k1):
                eng.dma_start(
                    out=wqk_sb[:, k0:k1, ct * 128:(ct + 1) * 128],
                    in_=wqk_r[:, k0:k1, ct * 128:(ct + 1) * 128],
                )

            def dma_x(eng, nb, k0, k1):
                eng.dma_start(
                    out=xT_sb[:, k0:k1, nb * 512:(nb + 1) * 512],
                    in_=xT_r[:, k0:k1, nb * 512:(nb + 1) * 512],
                )

            dma_wqk(nc.sync, 2, 0, 4)
            dma_x(nc.sync, 0, 0, 4)
            dma_wqk(nc.scalar, 2, 4, 8)
            dma_x(nc.scalar, 0, 4, 8)
            dma_wqk(nc.scalar, 0, 0, 8)
            nc.scalar.dma_start(out=wv_sb[:], in_=wv_r[:])
            dma_wqk(nc.sync, 3, 0, 8)
            dma_x(nc.sync, 1, 0, 8)
            dma_wqk(nc.sync, 1, 0, 8)
            dma_x(nc.sync, 2, 0, 8)
            dma_x(nc.sync, 3, 0, 8)
            nc.sync.dma_start(out=wp_sb[:], in_=wp_r[:])

            # --- constant/init memsets (after the DMA issues so they don't
            # delay the vector queue's dma_starts) ---------------------------
            nc.vector.memset(v_sb[:], 1.0)
            # select matrix for the denominator broadcast matmul:
            # out[m, q] = r[0, q] for m < 64 else r[32, q]
            sel_sb = consts.tile([33, 128], BF16, tag="sel")
            nc.vector.memset(sel_sb[:], 0.0)
            nc.vector.memset(sel_sb[0:1, 0:64], 1.0)
            nc.vector.memset(sel_sb[32:33, 64:128], 1.0)
            # persistent double-buffered denominator/reciprocal tiles
            # (init to 1.0 so the unused rows never hold NaN/Inf)
            d_rt = sbuf.tile([33, 2, 512], F32, tag="d")
            r_rt = sbuf.tile([33, 2, 512], F32, tag="r")
            rb_rt = sbuf.tile([33, 2, 512], BF16, tag="rb")
            nc.vector.memset(d_rt[:], 1.0)
            nc.vector.memset(r_rt[:], 1.0)
            nc.vector.memset(rb_rt[:], 1.0)

            # --- helper emitters -------------------------------------------
            def a_chain(pool, ct, nb):
                acc = pool.tile([128, 512], F32, tag="ab")
                for kt in range(KT):
                    nc.tensor.matmul(
                        acc[:],
                        lhsT=wqk_sb[:, kt, ct * 128:(ct + 1) * 128],
                        rhs=xT_sb[:, kt, nb * 512:(nb + 1) * 512],
                        start=(kt == 0),
                        stop=(kt == KT - 1),
                    )
                nc.vector.tensor_copy(
                    qk_sb[:, ct, nb * 512:(nb + 1) * 512], acc[:]
                )

            def b_chain(pool, nt):
                acc = pool.tile([128, 512], F32, tag="ab")
                for kt in range(KT):
                    nc.tensor.matmul(
                        acc[:, 0:GC],
                        lhsT=xT_sb[:, kt, nt * 128:(nt + 1) * 128],
                        rhs=wv_sb[:, kt, :],
                        start=(kt == 0),
                        stop=(kt == KT - 1),
                    )
                for h in range(HPG):
                    nc.vector.tensor_copy(
                        v_sb[:, nt, h * 65:h * 65 + 64],
                        acc[:, h * 64:(h + 1) * 64],
                    )

            def s_exp_block(psS, nqb, pair, e_t, nkts=None):
                qt = qk_sb[:, pair, :]
                kt_sb = qk_sb[:, 2 + pair, :]
                for nkt in (range(NKT) if nkts is None else nkts):
                    st = psS.tile([128, 1024], F32, tag="st")
                    for hh in range(2):
                        nc.tensor.matmul(
                            st[:, hh * 512:(hh + 1) * 512],
                            lhsT=kt_sb[
                                hh * 64:(hh + 1) * 64,
                                nkt * 128:(nkt + 1) * 128,
                            ],
                            rhs=qt[
                                hh * 64:(hh + 1) * 64,
                                nqb * 512:(nqb + 1) * 512,
                            ],
                            start=True,
                            stop=True,
                        )
                    nc.scalar.activation(
                        e_t[:, nkt, :],
                        st[:],
                        mybir.ActivationFunctionType.Exp,
                        scale=SCALE,
                    )

            def u_quarter(u_a, u_b, pair, e_t, nkts):
                # U_aug = [v|1]^T E^T per head with a padded 128-col
                # stationary window: rows 0-63 the head's v, row 64 the
                # softmax denominator, rows 65+ garbage (ignored).
                for nkt in nkts:
                    for hh, u_t in ((0, u_a), (1, u_b)):
                        h = pair * 2 + hh
                        nc.tensor.matmul(
                            u_t[:],
                            lhsT=v_sb[:, nkt, h * 65:h * 65 + 128],
                            rhs=e_t[:, nkt, hh * 512:(hh + 1) * 512],
                            start=(nkt == 0),
                            stop=(nkt == NKT - 1),
                        )

            def norm_part1(u_a, u_b, upool, k):
                # eager eviction: move the useful rows to SBUF so the PSUM
                # banks free immediately.  SBUF operands of a DVE op must
                # share the start partition and be 32-aligned, so head b
                # lands at partitions 64-127 and the denominators at rows
                # 0/32 of the double-buffered [33,2,512] tiles.
                u_sb = upool.tile([128, 512], F32, tag="usb")
                nc.vector.tensor_copy(u_sb[0:64, :], u_a[0:64, :])
                nc.vector.tensor_copy(u_sb[64:128, :], u_b[0:64, :])
                s = k % 2
                nc.vector.tensor_copy(d_rt[0:1, s, :], u_a[64:65, :])
                nc.vector.tensor_copy(d_rt[32:33, s, :], u_b[64:65, :])
                nc.vector.reciprocal(r_rt[:, s, :], d_rt[:, s, :])
                nc.vector.tensor_copy(rb_rt[:, s, :], r_rt[:, s, :])
                return u_sb

            def norm_part2(u_sb, pool, k, nqb, pair):
                # broadcast 1 -> 64 partitions with a tiny select matmul
                # (K=33: rows 1-31 of sel are zero, r rows 1-31 are 1.0).
                # Emitted one iteration after part1 so the PE FIFO never
                # waits on the 3.3us reciprocal.
                s = k % 2
                rr_ps = pool.tile([128, 512], F32, tag="ab")
                nc.tensor.matmul(
                    rr_ps[:], lhsT=sel_sb[:], rhs=rb_rt[:, s, :],
                    start=True, stop=True,
                )
                nc.vector.tensor_mul(
                    o_sb[0:64, pair, nqb * 512:(nqb + 1) * 512],
                    u_sb[0:64, :],
                    rr_ps[0:64, :],
                )
                nc.vector.tensor_mul(
                    o_sb[64:128, pair, nqb * 512:(nqb + 1) * 512],
                    u_sb[64:128, :],
                    rr_ps[64:128, :],
                )

            def e_block(pool, opool, nqb, mts=None):
                # partial output projection for query block nqb
                for mt in (range(nqb * 4, nqb * 4 + 4) if mts is None
                           else mts):
                    ot = opool.tile([128, DIM], BF16, tag="ot")
                    for nh in range(2):
                        acc = pool.tile([128, 512], F32, tag="ab")
                        for pair in range(2):
                            nc.tensor.matmul(
                                acc[:],
                                lhsT=o_sb[:, pair, mt * 128:(mt + 1) * 128],
                                rhs=wp_sb[:, pair, nh * 512:(nh + 1) * 512],
                                start=(pair == 0),
                                stop=(pair == 1),
                            )
                        nc.vector.tensor_copy(
                            ot[:, nh * 512:(nh + 1) * 512], acc[:]
                        )
                    nc.sync.dma_start(
                        out=out[mt * 128:(mt + 1) * 128, :], in_=ot[:]
                    )

            # --- wave schedule ---------------------------------------------
            WAVES = [(0, 0), (1, 0), (0, 1), (1, 1),
                     (2, 0), (2, 1), (3, 0), (3, 1)]

            with (
                tc.tile_pool(name="psS", bufs=2, space="PSUM") as psS,
                tc.tile_pool(name="psU", bufs=2, space="PSUM") as psU,
                tc.tile_pool(name="psAB", bufs=2, space="PSUM") as psAB,
                tc.tile_pool(name="epool", bufs=3) as epool,
                tc.tile_pool(name="upool", bufs=2) as upool,
                tc.tile_pool(name="opool", bufs=3) as opool,
            ):
                e_q = []

                def new_e():
                    e_t = epool.tile([128, NKT, 1024], BF16, tag="e")
                    e_q.append(e_t)
                    return e_t

                # lazily-allocated U accumulators and norm state per wave
                u_t = {}
                u_sbs = {}

                def uq(k, q):
                    if k not in u_t:
                        u_t[k] = (psU.tile([128, 512], F32, tag="u",
                                           name=f"u_a{k}"),
                                  psU.tile([128, 512], F32, tag="u",
                                           name=f"u_b{k}"))
                    u_a, u_b = u_t[k]
                    u_quarter(u_a, u_b, WAVES[k][1], e_q[k],
                              range(q * 4, q * 4 + 4))

                def p1(k):
                    u_a, u_b = u_t[k]
                    u_sbs[k] = norm_part1(u_a, u_b, upool, k)

                def p2(k):
                    norm_part2(u_sbs[k], psAB, k, *WAVES[k])

                def A(ct, nb):
                    return lambda: a_chain(psAB, ct, nb)

                def Bc(nt):
                    return lambda: b_chain(psAB, nt)

                def E(nqb, m0, m1):
                    return lambda: e_block(psAB, opool, nqb,
                                           mts=range(m0, m1))

                def U(k, q):
                    return lambda: uq(k, q)

                def P1(k):
                    return lambda: p1(k)

                def P2(k):
                    return lambda: p2(k)

                # segment table: seg[j] = (pre-ops, per-quarter filler ops,
                # post-ops).  S(w_j) quarters are emitted between the
                # fillers; the FIFO position of each op approximates its
                # execution wave, so U runs at lag 3 early (v/B not ready
                # sooner), catching down to lag ~0 by the last wave, and
                # norm chains (p1 -> p2 one segment later) never block.
                seg = [
                    ([A(2, 0), Bc(0), A(0, 0), Bc(1)],
                     [[A(2, 1), Bc(2)], [A(2, 2), Bc(3)],
                      [A(2, 3), Bc(4)], [A(0, 1), Bc(5)]], []),
                    ([],
                     [[A(3, 0), Bc(6)], [A(3, 1), Bc(7)],
                      [A(3, 2), Bc(8)], [A(3, 3), Bc(9)]], []),
                    ([A(1, 0)],
                     [[Bc(10)], [Bc(11), Bc(12)],
                      [A(1, 1), Bc(13)], [Bc(14), Bc(15)]], []),
                    ([],
                     [[U(0, 0), A(0, 2)], [U(0, 1)],
                      [U(0, 2)], [U(0, 3)]], [P1(0)]),
                    ([],
                     [[U(1, 0), A(1, 2)], [U(1, 1), P2(0)],
                      [U(1, 2)], [U(1, 3)]], [P1(1)]),
                    ([],
                     [[U(2, 0), A(0, 3)], [U(2, 1), P2(1)],
                      [U(2, 2)], [U(2, 3)]],
                     [P1(2), U(3, 0), U(3, 1)]),
                    ([],
                     [[U(3, 2), A(1, 3)], [U(3, 3), P2(2)],
                      [U(4, 0), U(4, 1)], [U(4, 2), U(4, 3)]],
                     [P1(3), P1(4), E(0, 0, 2)]),
                    ([],
                     [[U(5, 0), P2(3), E(0, 2, 3)],
                      [U(5, 1), U(5, 2), E(0, 3, 4)],
                      [U(5, 3), P2(4), E(1, 4, 5)],
                      [P1(5), U(6, 0), U(6, 1), E(1, 5, 6)]],
                     []),
                ]
                for j in range(8):
                    pre, quarters, post = seg[j]
                    for f in pre:
                        f()
                    e_j = new_e()
                    nqb, pair = WAVES[j]
                    for q in range(4):
                        # ramp segments: S quarter first, fillers slide in
                        # behind it; steady segments: U/norm fillers first
                        # so they sit ahead of the exp-paced S tiles.
                        if j < 3:
                            s_exp_block(psS, nqb, pair, e_j,
                                        nkts=range(q * 4, q * 4 + 4))
                        for f in quarters[q]:
                            f()
                        if j >= 3:
                            s_exp_block(psS, nqb, pair, e_j,
                                        nkts=range(q * 4, q * 4 + 4))
                    for f in post:
                        f()
                # tail: finish u(w6), u(w7) (tile-behind the last exps),
                # then the remaining norms and output projections.
                uq(6, 2)
                uq(6, 3)
                p1(6)
                e_block(psAB, opool, 1, mts=range(6, 8))
                p2(5)
                e_block(psAB, opool, 2, mts=range(8, 10))
                uq(7, 0)
                uq(7, 1)
                e_block(psAB, opool, 2, mts=range(10, 12))
                uq(7, 2)
                uq(7, 3)
                p1(7)
                p2(6)
                p2(7)
                e_block(psAB, opool, 3)

    _split_multi_waits(nc)
    return nc


_NC_CACHE = None


def _get_nc():
    global _NC_CACHE
    if _NC_CACHE is None:
        _NC_CACHE = build_core_kernel()
    return _NC_CACHE


def kernel(x, importance_weights, W_qkv, W_proj, b_proj, persistence_bias,
           _results_hook=None):
    x = np.asarray(x)
    W_qkv = np.asarray(W_qkv, dtype=np.float32)
    W_proj = np.asarray(W_proj, dtype=np.float32)
    b_proj = np.asarray(b_proj, dtype=np.float32)

    bf = ml_dtypes.bfloat16
    Q = W_qkv[:, 0:DIM]
    K = W_qkv[:, DIM:2 * DIM]
    V = W_qkv[:, 2 * DIM:3 * DIM]

    in_maps = []
    for core in range(8):
        b, g = divmod(core, 4)
        sl = slice(g * GC, (g + 1) * GC)
        in_maps.append({
            "xT": np.ascontiguousarray(x[b].T).astype(bf),
            "wqk": np.ascontiguousarray(
                np.concatenate([Q[:, sl], K[:, sl]], axis=1)).astype(bf),
            "wv": np.ascontiguousarray(V[:, sl]).astype(bf),
            "wp": np.ascontiguousarray(W_proj[sl, :]).astype(bf),
        })

    nc = _get_nc()
    res = run_bass_kernel_spmd(nc, in_maps, list(range(8)))
    if _results_hook is not None:
        _results_hook(res)

    out = np.zeros((B, N, DIM), dtype=np.float32)
    for core in range(8):
        b = core // 4
        out[b] += res.results[core]["out"].astype(np.float32)
    out += b_proj[None, None, :]
    return out
